# revision 77
# baseline (speedup 1.0000x reference)
"""Linear-attention MultiHeadAttentionBlock kernel for 8 Trainium2 NeuronCores.

Sharding: core c handles (batch b = c//2, head-group g = c%2).  Each core
computes, for its batch's q/k/v and its 8 heads (512 of the 1024 d_model
dims):
    QfT = elu(Wq_g @ X_q^T)+1          (transposed space: d' on partitions)
    Kf  = elu(X_k @ Wk_g^T)+1          (n-space)
    ksum= ones^T-stationary accumulate over n-tiles -> (2, 512) -> 4 PE
          transposes -> ksumS[dt] (128, 8) masked stationaries
    Vp  = X_v @ Wv_g^T                 (n-space)
    KV  = per head-pair dt: Kf_dt^T @ Vp_dt (128x128, PE-accumulated over n)
    Zpre= ksumS[dt]-stationary matmuls vs QfT -> (8, L) head-major
    zr  = 1/Zpre (DVE), partition-broadcast via an HBM round-trip
          (stride-0 DRAM-source DMAs)
    kvcat = KV * block-diag mask (DVE)
    outT = (kvcat_dt^T @ QfT) * zr     (m'-space; the normalize is
          hybrid: half the tiles fuse zr into the DVE PSUM->SBUF copy,
          half pre-scale QfT and take a plain ACT copy, so both engines
          share the V->final boundary work)
    y    = WoS @ out_g^T               (partial d_model-1024 output, bf16)
Host upcasts and sums the two per-batch partials.

All big-matmul operands are bf16: halves HBM traffic vs fp32, and bf16
moving operands stream ~2x faster through the PE than fp32r on real HW.
PSUM accumulation stays fp32.  The Z chain runs in f32r/f32.

Structure notes (measured on HW, do not regress):
  - x/w inputs DMA as one (128, 8, L) tile per tensor in a few big pieces
    (HWDGE descriptor-gen serializes at ~0.6us per DMA instruction).
  - ksA ksum matmuls and KV matmuls are staggered one n-tile behind the
    projections so the PE never parks on a feature-map/Vp-copy chain.
  - Feature-map temps must stay f32: bf16 activation IO regressed HW by
    ~45us (activation-table or conversion penalty on ACT).
  - The Z chain (Zpre -> recip -> HBM write -> broadcast DMAs) is
    spread across the V-phase n-tile loop; the normalize lands in the
    outT copy, so outT's matmuls never wait on the zr chain.
    SBUF-source stride-0 DMA is rejected, DRAM-source is allowed --
    hence the scratch round-trip.
"""

import numpy as np

import concourse.bass as bass
import concourse.mybir as mybir
import concourse.tile as tile
from concourse import bacc
from concourse.bass_utils import run_bass_kernel_spmd
from concourse.masks import make_identity

P = 128
L = 2048          # sequence length
DM = 1024         # d_model (= contraction dim of projections)
DG = 512          # per-core head-group width (8 heads x 64)
NT = L // P       # 16 n-tiles
KC = DM // P      # 8 contraction chunks
DT = DG // P      # 4 d'-tiles (2 heads each)
NCH = 4           # n-chunks of 512
F32 = mybir.dt.float32
BF16 = mybir.dt.bfloat16

_CACHE = {}


def build_nc(repeats=1):
    nc = bacc.Bacc(None, target_bir_lowering=False)

    xq_d = nc.dram_tensor("xqT", [DM, L], BF16, kind="ExternalInput")
    xk_d = nc.dram_tensor("xkT", [DM, L], BF16, kind="ExternalInput")
    xv_d = nc.dram_tensor("xvT", [DM, L], BF16, kind="ExternalInput")
    wq_d = nc.dram_tensor("wqT", [DM, DG], BF16, kind="ExternalInput")
    wk_d = nc.dram_tensor("wkT", [DM, DG], BF16, kind="ExternalInput")
    wv_d = nc.dram_tensor("wvT", [DM, DG], BF16, kind="ExternalInput")
    wo_d = nc.dram_tensor("woT", [DG, DM], BF16, kind="ExternalInput")
    y_d = nc.dram_tensor("y", [DM, L], BF16, kind="ExternalOutput")
    # HBM scratch for the zr partition-broadcast round-trip
    zrh_d = nc.dram_tensor("zr_scratch", [8, L], mybir.dt.float32r,
                           kind="ExternalOutput")

    with tile.TileContext(nc) as tc:
        with (
            tc.tile_pool(name="const", bufs=1) as cpool,
            tc.tile_pool(name="xt", bufs=2) as xt,       # (128,8,2048) x tensors
            tc.tile_pool(name="wt", bufs=2) as wt,       # (128,8,512) weights
            tc.tile_pool(name="wo", bufs=1) as wop,      # (128,4,1024) w_o
            tc.tile_pool(name="qft", bufs=16) as qftp,   # QfT persistent
            tc.tile_pool(name="kf", bufs=16) as kfp,     # Kf, later outT
            tc.tile_pool(name="vp", bufs=4) as vpp,      # Vp rotating
            tc.tile_pool(name="tmp", bufs=10) as tmp,    # feature-map temps
            tc.tile_pool(name="zrs", bufs=4) as zrsp,    # (128,2048) zr bcast
            tc.tile_pool(name="misc", bufs=1) as misc,   # zrA/ksumS/kvcat/ksA_sb
            tc.tile_pool(name="ysb", bufs=2) as ysb,     # (128,2048) y row-batches
            tc.tile_pool(name="pp", bufs=4, space="PSUM") as pp,
            tc.tile_pool(name="kvp", bufs=4, space="PSUM") as kvp,
        ):
            ident = cpool.tile([P, P], F32, name="ident")
            make_identity(nc, ident[:])
            ones_f = cpool.tile([P, 2], F32, name="ones_f")
            nc.gpsimd.memset(ones_f[:], 1.0)
            ones2 = cpool.tile([P, 2], BF16, name="ones2")
            nc.vector.tensor_copy(ones2[:], ones_f[:])
            # block-diag (128,128) mask: 1 where (i<64)==(j<64)
            bm_f = cpool.tile([P, P], F32, name="bm_f")
            nc.gpsimd.memset(bm_f[:], 0.0)
            nc.gpsimd.memset(bm_f[0:64, 0:64], 1.0)
            nc.gpsimd.memset(bm_f[64:128, 64:128], 1.0)
            blkmask = cpool.tile([P, P], BF16, name="blkmask")
            nc.vector.tensor_copy(blkmask[:], bm_f[:])
            for _rep in range(repeats):
                body(nc, tc, ident, ones2, blkmask, cpool,
                     xt, wt, wop, qftp, kfp, vpp, tmp, zrsp, misc, ysb,
                     pp, kvp,
                     xq_d, xk_d, xv_d, wq_d, wk_d, wv_d, wo_d, zrh_d, y_d)

    nc.compile()
    return nc


def body(nc, tc, ident, ones2, blkmask, cpool,
         xt, wt, wop, qftp, kfp, vpp, tmp, zrsp, misc, ysb, pp, kvp,
         xq_d, xk_d, xv_d, wq_d, wk_d, wv_d, wo_d, zrh_d, y_d):
    Exp = mybir.ActivationFunctionType.Exp
    Relu = mybir.ActivationFunctionType.Relu
    Alu = mybir.AluOpType

    def feature_map(ps, dst):
        # dst = elu(ps)+1 = exp(min(ps,0)) + relu(ps)
        # (temps must stay f32 -- see module docstring)
        t0 = tmp.tile([P, 512], F32, tag="tmp", name="t0")
        t1 = tmp.tile([P, 512], F32, tag="tmp", name="t1")
        nc.vector.tensor_scalar(t0[:], ps[:], 0.0, None, Alu.min)
        nc.scalar.activation(t1[:], ps[:], Relu)
        nc.scalar.activation(dst[:], t0[:], Exp)
        nc.vector.tensor_tensor(dst[:], dst[:], t1[:], Alu.add)

    def dma_x_all(src_d, name, npieces=1):
        # all 8 chunks as one (128, 8, L) tile; optionally split the DMA
        # into kc-groups so early chunks land before the full transfer
        t = xt.tile([P, KC, L], BF16, tag="xa", name=name, bufs=2)
        src = src_d.rearrange("(c p) n -> p c n", p=P)
        step = KC // npieces
        for i in range(0, KC, step):
            nc.sync.dma_start(t[:, i:i + step, :], src[:, i:i + step, :])
        return t

    def dma_w_all(src_d, name, npieces=1):
        # all 8 weight chunks as one (128, 8, DG) tile
        t = wt.tile([P, KC, DG], BF16, tag="wt", name=name, bufs=2)
        src = src_d.rearrange("(c p) n -> p c n", p=P)
        step = KC // npieces
        for i in range(0, KC, step):
            nc.sync.dma_start(t[:, i:i + step, :], src[:, i:i + step, :])
        return t

    # ---------------- Phase Q: QfT (transposed space) ----------------
    # split + interleave the first x/w transfers so chunk kc=0 lands quickly
    xqa = xt.tile([P, KC, L], BF16, tag="xa", name="xq", bufs=2)
    wqa = wt.tile([P, KC, DG], BF16, tag="wt", name="wq", bufs=2)
    xq_src = xq_d.rearrange("(c p) n -> p c n", p=P)
    wq_src = wq_d.rearrange("(c p) n -> p c n", p=P)
    nc.sync.dma_start(wqa[:, 0:1, :], wq_src[:, 0:1, :])
    nc.sync.dma_start(xqa[:, 0:1, 0:1024], xq_src[:, 0:1, 0:1024])
    nc.sync.dma_start(xqa[:, 0:1, 1024:2048], xq_src[:, 0:1, 1024:2048])
    nc.sync.dma_start(wqa[:, 1:3, :], wq_src[:, 1:3, :])
    nc.sync.dma_start(xqa[:, 1:2, :], xq_src[:, 1:2, :])
    nc.sync.dma_start(wqa[:, 3:8, :], wq_src[:, 3:8, :])
    nc.sync.dma_start(xqa[:, 2:4, :], xq_src[:, 2:4, :])
    nc.sync.dma_start(xqa[:, 4:6, :], xq_src[:, 4:6, :])
    nc.sync.dma_start(xqa[:, 6:8, :], xq_src[:, 6:8, :])

    qftl = [None] * 16  # (128, 512) tiles: index dt*NCH + nch

    def qft(dt, nch):
        return qftl[dt * NCH + nch][:]

    for dt in range(DT):
        ypool, ytag = ((pp, "pp") if dt % 2 == 0 else (kvp, "acc"))
        psq = [ypool.tile([P, 512], F32, tag=ytag, name=f"psq{_n}")
               for _n in range(NCH)]
        for kc in range(KC):
            for nch in range(NCH):
                nc.tensor.matmul(
                    psq[nch][:],
                    wqa[:, kc, dt * P:(dt + 1) * P],
                    xqa[:, kc, nch * 512:(nch + 1) * 512],
                    start=(kc == 0), stop=(kc == KC - 1),
                )
        for nch in range(NCH):
            qf = qftp.tile([P, 512], BF16, tag="qft")
            feature_map(psq[nch], qf)
            qftl[dt * NCH + nch] = qf

    # ---------------- Phase K: Kf (n-space) + ksum ----------------
    xka = dma_x_all(xk_d, "xk")
    wka = dma_w_all(wk_d, "wk")
    kf = []
    ksA = kvp.tile([2, 512], F32, tag="acc", name="ksA")
    for nt in range(NT):
        ps = pp.tile([P, 512], F32, tag="pp")
        for kc in range(KC):
            nc.tensor.matmul(
                ps[:],
                xka[:, kc, nt * P:(nt + 1) * P],
                wka[:, kc, :],
                start=(kc == 0), stop=(kc == KC - 1),
            )
        kft = kfp.tile([P, 512], BF16, tag="kf")
        feature_map(ps, kft)
        kf.append(kft)
        # ksum accumulate: (2,512) += ones2^T @ Kf_(nt-1), staggered one
        # n-tile behind the projections so the PE never waits on the
        # feature-map chain
        if nt > 0:
            nc.tensor.matmul(
                ksA[:], ones2[:], kf[nt - 1][:],
                start=(nt == 1), stop=False,
            )

    def ksum_tail():
        # last ksA accumulate + ksum -> d'-partition masked stationaries
        # ksumS[dt] (128, 8) bf16.  Emitted early in phase V so the PE is
        # never parked on kf[15]'s feature-map chain.
        nc.tensor.matmul(ksA[:], ones2[:], kf[NT - 1][:],
                         start=False, stop=True)
        ksA_sb = misc.tile([2, 512], F32, tag="ksA_sb", name="ksA_sb")
        nc.scalar.copy(ksA_sb[:], ksA[0:2, :])
        ksumS = []
        for dt in range(DT):
            ztp = pp.tile([P, 2], F32, tag="pp", name="ztp")
            nc.tensor.transpose(ztp[:], ksA_sb[0:2, dt * P:(dt + 1) * P],
                                ident[0:2, 0:2])
            ks = misc.tile([P, 8], BF16, tag="ksumS", name=f"ksumS{dt}",
                           bufs=4)
            nc.gpsimd.memset(ks[:], 0.0)
            nc.scalar.copy(ks[0:64, 2 * dt:2 * dt + 1], ztp[0:64, 0:1])
            nc.scalar.copy(ks[64:128, 2 * dt + 1:2 * dt + 2],
                           ztp[64:128, 0:1])
            ksumS.append(ks)
        return ksumS

    # ---------------- Phase V: Vp + KV accumulation + Z chain ----------------
    xva = dma_x_all(xv_d, "xv")
    wva = dma_w_all(wv_d, "wv")
    woa = wop.tile([P, DT, DM], BF16, tag="wo", name="wo_t", bufs=1)
    nc.sync.dma_start(woa[:], wo_d.rearrange("(c p) n -> p c n", p=P))

    kvt = [kvp.tile([P, P], F32, tag="acc", name=f"kvt{_dt}")
           for _dt in range(DT)]
    zrs = [None] * DT
    zp = [None] * NCH
    zrA = misc.tile([8, L], mybir.dt.float32r, tag="zrA", name="zrA")

    def kv_mms(nt):
        vt, kft = vps[nt % 3], kf[nt]
        for dt in range(DT):
            nc.tensor.matmul(
                kvt[dt][:],
                kft[:, dt * P:(dt + 1) * P],
                vt[:, dt * P:(dt + 1) * P],
                start=(nt == 0), stop=(nt == NT - 1),
            )

    vps = [None] * 3
    for nt in range(NT):
        ps = pp.tile([P, 512], F32, tag="pp")
        for kc in range(KC):
            nc.tensor.matmul(
                ps[:],
                xva[:, kc, nt * P:(nt + 1) * P],
                wva[:, kc, :],
                start=(kc == 0), stop=(kc == KC - 1),
            )
        vp_t = vpp.tile([P, 512], BF16, tag="vp")
        nc.scalar.copy(vp_t[:], ps[:])
        vps[nt % 3] = vp_t
        if nt == 0:
            ksumS = ksum_tail()
        # KV matmuls staggered one n-tile behind the projections so the PE
        # never waits on the Vp copy
        if nt > 0:
            kv_mms(nt - 1)
        # interleaved Z chain (inputs qft/ksumS ready since phase Q/K)
        if 2 <= nt < 6:
            # Zpre group for n-chunk nt-2: accumulate 4 masked stationaries
            nch = nt - 2
            zp[nch] = pp.tile([8, 512], F32, tag="pp", name=f"zp{nch}")
            for dt in range(DT):
                nc.tensor.matmul(
                    zp[nch][:], ksumS[dt][:],
                    qft(dt, nch),
                    start=(dt == 0), stop=(dt == DT - 1),
                )
        elif 6 <= nt < 10:
            nch = nt - 6
            with nc.allow_low_precision(reason="zr in tf32 is plenty"):
                nc.vector.reciprocal(zrA[:, nch * 512:(nch + 1) * 512],
                                     zp[nch][:])
            nc.sync.dma_start(zrh_d[:, nch * 512:(nch + 1) * 512],
                              zrA[:, nch * 512:(nch + 1) * 512])
        elif 10 <= nt < 14:
            # zr partition-broadcast via DMA from the HBM scratch (frees
            # the PE of 16 f32r sel8 matmuls)
            dt = nt - 10
            zt = zrsp.tile([P, L], mybir.dt.float32r, tag="zrs",
                           name=f"zrs{dt}")
            nc.sync.dma_start(
                zt[0:64, :], zrh_d[2 * dt:2 * dt + 1, :].broadcast_to((64, L)))
            nc.sync.dma_start(
                zt[64:128, :],
                zrh_d[2 * dt + 1:2 * dt + 2, :].broadcast_to((64, L)))
            zrs[dt] = zt
        if nt >= 12:
            # pre-scale the QfT tiles whose outT copy runs on ACT (which
            # cannot fuse a multiply); the rest are normalized in the
            # fused DVE copy later
            dt = nt - 12
            for nch in range(NCH):
                if (nch + dt) % 2 == 1:
                    qt = qft(dt, nch)
                    nc.vector.tensor_tensor(
                        qt, qt, zrs[dt][:, nch * 512:(nch + 1) * 512],
                        Alu.mult)
    kv_mms(NT - 1)

    # kvcat[dt] = KV block-diagonal via mask (bf16 for the outT stationary)
    kvcat = []
    for dt in range(DT):
        kvc = misc.tile([P, P], BF16, tag="kvcat", bufs=4)
        nc.vector.tensor_tensor(kvc[:], kvt[dt][:], blkmask[:], Alu.mult)
        kvcat.append(kvc)

    # ---------------- transposed out ----------------
    # outT[(dt, nch)]: (128 m'-part, 512 n) = kvcat[dt]^T @ QfTz
    oTs = [[None] * DT for _ in range(NCH)]
    for dt in range(DT):
        for nch in range(NCH):
            otp = pp.tile([P, 512], F32, tag="pp", name="otp")
            nc.tensor.matmul(
                otp[:], kvcat[dt][:], qft(dt, nch),
                start=True, stop=True,
            )
            oT = kfp.tile([P, 512], BF16, tag="kf", name="oT")
            # hybrid normalize: even tiles fuse the zr multiply into the
            # DVE copy (exact: zr is constant within each head's
            # contraction block); odd tiles were pre-scaled in QfT and
            # take the plain ACT copy, keeping both engines busy at the
            # V->final boundary
            if (nch + dt) % 2 == 0:
                nc.vector.tensor_tensor(
                    oT[:], otp[:], zrs[dt][:, nch * 512:(nch + 1) * 512],
                    Alu.mult)
            else:
                nc.scalar.copy(oT[:], otp[:])
            oTs[nch][dt] = oT

    def outT(nch, dc):
        return oTs[nch][dc][:]

    # ---------------- final projection ----------------
    # yT = WoS @ out_g^T: wo blocks stationary, reused across the 4 n-chunks
    for jb in range(8):
        ypool, ytag = ((kvp, "acc") if jb % 2 == 0 else (pp, "pp"))
        yps = [ypool.tile([P, 512], F32, tag=ytag, name=f"yp{_n}")
               for _n in range(NCH)]
        yt = ysb.tile([P, L], BF16, tag="ysb", name="yt")
        if jb < 7:
            for dc in range(DT):
                for nch in range(NCH):
                    nc.tensor.matmul(
                        yps[nch][:],
                        woa[:, dc, jb * P:(jb + 1) * P],
                        outT(nch, dc),
                        start=(dc == 0), stop=(dc == DT - 1),
                    )
            for nch in range(NCH):
                if (jb + nch) % 2 == 0:
                    nc.vector.tensor_copy(
                        yt[:, nch * 512:(nch + 1) * 512], yps[nch][:])
                else:
                    nc.scalar.copy(
                        yt[:, nch * 512:(nch + 1) * 512], yps[nch][:])
            nc.sync.dma_start(y_d[jb * P:(jb + 1) * P, :], yt[:])
        else:
            # last row-block: nch-outer so each chunk's copy and DMA
            # overlap the remaining matmuls, shortening the kernel tail
            for nch in range(NCH):
                for dc in range(DT):
                    nc.tensor.matmul(
                        yps[nch][:],
                        woa[:, dc, jb * P:(jb + 1) * P],
                        outT(nch, dc),
                        start=(dc == 0), stop=(dc == DT - 1),
                    )
                if (jb + nch) % 2 == 0:
                    nc.vector.tensor_copy(
                        yt[:, nch * 512:(nch + 1) * 512], yps[nch][:])
                else:
                    nc.scalar.copy(
                        yt[:, nch * 512:(nch + 1) * 512], yps[nch][:])
                nc.sync.dma_start(
                    y_d[jb * P:(jb + 1) * P, nch * 512:(nch + 1) * 512],
                    yt[:, nch * 512:(nch + 1) * 512])


def make_in_maps(q, k, v, w_q, w_k, w_v, w_o):
    bf16 = mybir.dt.np(BF16)
    q = np.asarray(q, dtype=np.float32)
    k = np.asarray(k, dtype=np.float32)
    v = np.asarray(v, dtype=np.float32)
    w_q = np.asarray(w_q, dtype=np.float32)
    w_k = np.asarray(w_k, dtype=np.float32)
    w_v = np.asarray(w_v, dtype=np.float32)
    w_o = np.asarray(w_o, dtype=np.float32)
    B = q.shape[0]
    xqT = [np.ascontiguousarray(q[b].T).astype(bf16) for b in range(B)]
    xkT = [np.ascontiguousarray(k[b].T).astype(bf16) for b in range(B)]
    xvT = [np.ascontiguousarray(v[b].T).astype(bf16) for b in range(B)]
    wqT = [np.ascontiguousarray(w_q[g * DG:(g + 1) * DG, :].T).astype(bf16)
           for g in range(2)]
    wkT = [np.ascontiguousarray(w_k[g * DG:(g + 1) * DG, :].T).astype(bf16)
           for g in range(2)]
    wvT = [np.ascontiguousarray(w_v[g * DG:(g + 1) * DG, :].T).astype(bf16)
           for g in range(2)]
    woT = [np.ascontiguousarray(w_o[:, g * DG:(g + 1) * DG].T).astype(bf16)
           for g in range(2)]
    in_maps = []
    for c in range(8):
        b, g = c // 2, c % 2
        in_maps.append({
            "xqT": xqT[b], "xkT": xkT[b], "xvT": xvT[b],
            "wqT": wqT[g], "wkT": wkT[g], "wvT": wvT[g], "woT": woT[g],
        })
    return in_maps


def kernel(q, k, v, mask, w_q, w_k, w_v, w_o):
    if "nc" not in _CACHE:
        _CACHE["nc"] = build_nc()
    nc = _CACHE["nc"]
    in_maps = make_in_maps(q, k, v, w_q, w_k, w_v, w_o)
    res = run_bass_kernel_spmd(nc, in_maps, list(range(8)))
    _CACHE["last_results"] = res
    B = np.asarray(q).shape[0]
    out = np.empty((B, L, DM), dtype=np.float32)
    for b in range(B):
        out[b] = (res.results[2 * b]["y"].astype(np.float32)
                  + res.results[2 * b + 1]["y"].astype(np.float32)).T
    return out
